# revision 15
# baseline (speedup 1.0000x reference)
"""DeepGEMM-style fp8 linear on 8 TRN2 NeuronCores.

Computes: out = bf16( fp8(x_pad) @ (fp8(W) * block_scale).T ) + bias, sliced to
[16384, 4000], matching the jax reference (block scales are ones, bias zeros).

Strategy: batch-parallel SPMD with HOST-side fp8 quantization. The reference
stores x and w as fp8_e4m3; quantizing on host (exactly reproducing the
reference's e4m3fn rounding — exact under TRN2's e4m3 container for this
data range) means each core streams 1-byte operands: x shard 8.4MB + full
weight 16.8MB + out 16.8MB ≈ 42MB/core, entirely hidden under the PE work.
On device: fp8 matmul (DoubleRow perf mode) accumulating f32 in PSUM,
bias-add + bf16 cast on DVE, store out as [n, b]; host transposes back.

The kernel is PE-issue-bound at the fp8 DoubleRow roofline: 2048 matmuls/core
x 512 streamed columns ~= 437us at 2.4GHz (fp8 streams 1 column/cycle;
DoubleRow packs 2 k-subtiles per column; verified vs an 'onetile' probe and
the timeline simulator, both showing a dependency-stall-free PE stream).
x is split across both HWDGE rings, bias rides the idle Pool SWDGE, out
chunks store per-512-column group, psum_bufs=8/wq_bufs=3 give cross-tile
ILP (worth 25us/5us per the timeline sim).
"""

import sys

if "/opt/trn_rl_repo" not in sys.path:
    sys.path.insert(0, "/opt/trn_rl_repo")

import numpy as np
import ml_dtypes

P = 128
N_CORES = 8
BATCH = 16384
IN_F = 4000
OUT_F = 4000
K_PAD = 4096               # in-features padded to 32 k-subtiles of 128
N_PAD = 4096               # out-features padded 4032 -> 4096 (uniform n-tiles)

_kernel_cache = {}

# test.py knobs
TRACE = False
LAST_RESULTS = None


def _build(b_sh, ks, nt, bg, double_row, reps=1, ramp_nt=0, probe="none",
           wq_bufs=3, out_bufs=3, psum_bufs=8, x_split=True, out_per_bg=True,
           head_split=False):
    import contextlib
    from concourse import bacc, tile, mybir
    from concourse.mybir import dt

    nbg = b_sh // bg
    assert nbg * bg == b_sh
    kk = ks // 2 if double_row else ks
    nc = bacc.Bacc(None, target_bir_lowering=False, debug=False)

    with tile.TileContext(nc) as tc:
        with tc.tile_pool(name="dram", bufs=1, space="DRAM") as dram:
            xt = dram.tile([ks // 2, P, 2, b_sh], dt.float8e4,
                           kind="ExternalInput", name="xt", uniquify=False)
            wp = dram.tile([nt, P, ks, P], dt.float8e4,
                           kind="ExternalInput", name="wp", uniquify=False)
            bvec = dram.tile([P, nt], dt.bfloat16, kind="ExternalInput",
                             name="bvec", uniquify=False)
            out = dram.tile([nt, P, b_sh], dt.bfloat16, kind="ExternalOutput",
                            name="out", uniquify=False)

        with tc.tile_pool(name="const", bufs=1) as const, \
             tc.tile_pool(name="xqp", bufs=1) as xqp, \
             tc.tile_pool(name="wqp", bufs=wq_bufs) as wqp, \
             tc.tile_pool(name="prtp", bufs=max(ramp_nt, 1)) as prtp, \
             tc.tile_pool(name="outp", bufs=out_bufs) as outp, \
             tc.tile_pool(name="psp", bufs=psum_bufs, space="PSUM") as psp, \
             (tc.For_i(0, reps, 1) if reps > 1
              else contextlib.nullcontext()):

            # bias: [P, nt] bf16 -> f32 (per-partition scalars, col = n-tile).
            # On the otherwise-idle Pool SWDGE so it never queues ahead of the
            # critical first x/w chunks on the HWDGE rings; not needed until
            # the first epilogue ~14us in.
            bias_bf = const.tile([P, nt], dt.bfloat16)
            nc.gpsimd.dma_start(out=bias_bf[:, :], in_=bvec[:, :])
            bias_sb = const.tile([P, nt], dt.float32)
            nc.vector.tensor_copy(bias_sb[:, :], bias_bf[:, :])

            def load_w(n, pool, split_head=False):
                # weight n-tile fp8 [P, ks, P]: 4KB contiguous per partition,
                # on scalar's HWDGE ring so w doesn't queue behind x/out
                wq = pool.tile([P, ks, P], dt.float8e4, name="wq")
                if split_head:
                    # first k-pair lands in a small early DMA so matmul #0
                    # isn't gated on the full 4KB/partition tile
                    nc.scalar.dma_start(out=wq[:, 0:2, :], in_=wp[n][:, 0:2, :])
                    nc.scalar.dma_start(out=wq[:, 2:, :], in_=wp[n][:, 2:, :])
                else:
                    nc.scalar.dma_start(out=wq[:, :, :], in_=wp[n])
                return wq

            # first weight tiles before the x odd-pairs claim the scalar ring
            pre_wq = {}
            if probe != "peonly" and x_split and ramp_nt == 0:
                for n in range(min(wq_bufs - 1, nt)):
                    pre_wq[n] = load_w(n, wqp, split_head=(head_split and n == 0))

            # x: fp8 straight from HBM into one resident tile, k-pair per DMA
            # (4KB contiguous per partition line); split pairs across the two
            # HWDGE rings so the stream lands in half the time
            xq = xqp.tile([P, ks, b_sh], dt.float8e4)
            for k2 in range(ks // 2):
                eng = nc.scalar if (x_split and k2 % 2) else nc.sync
                if head_split and k2 == 0:
                    # chunk the first pair so matmul (k=0, g) only waits for
                    # its own bg slice (~1KB/partition) instead of the pair
                    for g in range(nbg):
                        eng.dma_start(
                            out=xq[:, 0:2, g * bg:(g + 1) * bg],
                            in_=xt[0][:, :, g * bg:(g + 1) * bg])
                else:
                    eng.dma_start(out=xq[:, 2 * k2:2 * k2 + 2, :], in_=xt[k2])

            def mk_mm(wq):
                def mm(ps, g, k, start, stop):
                    if probe == "onetile":
                        k, g = 0, 0
                    if double_row:
                        nc.tensor.matmul(
                            ps[:, :],
                            lhsT=wq[:, 2 * k:2 * k + 2, :],
                            rhs=xq[:, 2 * k:2 * k + 2, g * bg:(g + 1) * bg],
                            start=start, stop=stop,
                            perf_mode=mybir.MatmulPerfMode.DoubleRow)
                    else:
                        nc.tensor.matmul(
                            ps[:, :],
                            lhsT=wq[:, k, :],
                            rhs=xq[:, k, g * bg:(g + 1) * bg],
                            start=start, stop=stop)
                return mm

            wq0 = None
            if probe == "peonly":
                # single weight tile loaded once: removes the w stream
                wq0 = load_w(0, wqp)

            # Ramp phase: while x is still streaming in, run the first-half-K
            # accumulation for the first ramp_nt n-tiles so the PE isn't
            # gated on the last k-subtile's arrival; partials park in SBUF.
            half = kk // 2
            ramp_wq, ramp_part = {}, {}
            for n in range(ramp_nt):
                rwq = load_w(n, wqp)
                ramp_wq[n] = rwq
                part = prtp.tile([P, b_sh], dt.float32, name="part")
                ramp_part[n] = part
                mm = mk_mm(rwq)
                pss = [psp.tile([P, bg], mybir.dt.float32, name="ps")
                       for _ in range(nbg)]
                for k in range(half):
                    for g in range(nbg):
                        mm(pss[g], g, k, k == 0, k == half - 1)
                for g in range(nbg):
                    nc.vector.tensor_copy(
                        part[:, g * bg:(g + 1) * bg], pss[g][:, :])

            for n in range(nt):
                ramp = n < ramp_nt
                if probe == "peonly":
                    wq = wq0
                elif ramp:
                    wq = ramp_wq[n]
                elif n in pre_wq:
                    wq = pre_wq[n]
                else:
                    wq = load_w(n, wqp)

                out_sb = outp.tile([P, b_sh], dt.bfloat16, name="out_sb")
                mm = mk_mm(wq)
                k_lo = half if ramp else 0

                def epilogue(g, ps):
                    dst = out_sb[:, g * bg:(g + 1) * bg]
                    if ramp:
                        # (psum + bias) + first-half partial -> bf16
                        nc.vector.scalar_tensor_tensor(
                            dst, ps[:, :], bias_sb[:, n:n + 1],
                            ramp_part[n][:, g * bg:(g + 1) * bg],
                            mybir.AluOpType.add, mybir.AluOpType.add)
                    else:
                        nc.vector.tensor_scalar_add(
                            dst, ps[:, :], bias_sb[:, n:n + 1])

                if probe == "noMM":
                    # one MM per psum tile: PE work ~1/16th, rest identical
                    pss = [psp.tile([P, bg], mybir.dt.float32, name="ps")
                           for _ in range(nbg)]
                    for g in range(nbg):
                        mm(pss[g], g, 0, True, True)
                    for g in range(nbg):
                        epilogue(g, pss[g])
                else:
                    # consecutive MMs share the stationary tile -> weight
                    # loads amortize/hide across nbg matmuls
                    pss = [psp.tile([P, bg], mybir.dt.float32, name="ps")
                           for _ in range(nbg)]
                    for k in range(k_lo, kk):
                        for g in range(nbg):
                            mm(pss[g], g, k, k == k_lo, k == kk - 1)
                    for g in range(nbg):
                        epilogue(g, pss[g])
                        if out_per_bg:
                            # store each bg chunk as soon as its epilogue is
                            # done -> shorter drain tail on the last tile
                            nc.sync.dma_start(
                                out=out[n][:, g * bg:(g + 1) * bg],
                                in_=out_sb[:, g * bg:(g + 1) * bg])

                if not out_per_bg or probe == "noMM":
                    nc.sync.dma_start(out=out[n], in_=out_sb[:, :])

    nc.finalize()
    return nc


def _get_nc(key):
    if key not in _kernel_cache:
        _kernel_cache[key] = _build(*key)
    return _kernel_cache[key]


F8 = ml_dtypes.float8_e4m3          # TRN2's fp8e4 container
F8REF = ml_dtypes.float8_e4m3fn     # reference's quantization format


def kernel(x, weight, weight_scale, bias):
    global LAST_RESULTS
    from concourse.bass_utils import run_bass_kernel_spmd

    x = np.asarray(x, dtype=np.float32)
    weight = np.asarray(weight, dtype=np.float32)
    weight_scale = np.asarray(weight_scale, dtype=np.float32)
    bias = np.asarray(bias)  # bf16

    n_out, k_pad = weight.shape          # 4032, 4096
    batch, in_f = x.shape                # 16384, 4000
    assert k_pad == K_PAD and batch == BATCH

    b_sh = batch // N_CORES
    ks = K_PAD // P
    nt = N_PAD // P
    bg = 512

    # Quantize exactly like the reference (e4m3fn round-to-nearest), then
    # recast to the e4m3 container TRN2 uses — exact for values in range.
    wq8 = weight.astype(F8REF).astype(F8)
    if not np.allclose(weight_scale, 1.0):
        # best-effort fold of non-unit block scales (spec ships ones)
        ws = np.repeat(np.repeat(weight_scale, P, axis=0), P, axis=1)
        wdq = weight.astype(F8REF).astype(np.float32) * ws[:n_out, :k_pad]
        wq8 = wdq.astype(F8)

    # w -> [nt, p, ks, j]: element = w[nt*128 + j, ks*128 + p], zero-pad rows
    wpad = np.zeros((N_PAD, K_PAD), dtype=F8)
    wpad[:n_out] = wq8
    wp = np.ascontiguousarray(
        wpad.reshape(nt, P, ks, P).transpose(0, 3, 2, 1))

    # bias -> [p, nt] bf16, zero-padded
    bpad = np.zeros(N_PAD, dtype=ml_dtypes.bfloat16)
    bpad[:n_out] = bias
    bvec = np.ascontiguousarray(bpad.reshape(nt, P).T)

    xq8 = x.astype(F8REF).astype(F8)     # [batch, in_f] fp8
    in_maps = []
    for c in range(N_CORES):
        shard = xq8[c * b_sh:(c + 1) * b_sh]        # [b_sh, in_f]
        xt = np.zeros((K_PAD, b_sh), dtype=F8)
        xt[:in_f] = shard.T
        in_maps.append({
            "xt": xt.reshape(ks // 2, 2, P, b_sh).transpose(0, 2, 1, 3).copy(),
            "wp": wp,
            "bvec": bvec,
        })

    global _last_in_maps
    _last_in_maps = in_maps
    nc = _get_nc((b_sh, ks, nt, bg, True, 1))
    res = run_bass_kernel_spmd(nc, in_maps, list(range(N_CORES)), trace=TRACE)
    LAST_RESULTS = res

    final = np.empty((batch, OUT_F), dtype=ml_dtypes.bfloat16)
    for c in range(N_CORES):
        oc = res.results[c]["out"].reshape(N_PAD, b_sh)
        final[c * b_sh:(c + 1) * b_sh, :] = oc[:OUT_F].T
    return final
